# revision 41
# baseline (speedup 1.0000x reference)
"""Multi-head attention (B=4, S=2048, H=8, Dh=64, Dm=512) on 8 TRN2 NeuronCores.

Sharding: batch*head parallel. Core c owns batch b = c//2 and head group
g = c%2 (4 heads each). Each core computes QKV projection for its head
group, transposed-scores flash-style attention (no max subtraction --
scores ~ N(0,1) after 1/sqrt(Dh) scaling), and its partial output
projection against its 256 rows of Wo. The host sums the two partial
projections per batch.

Optimizations vs the first working kernel (194-230us -> ~174us):
  - Whole datapath in fp16 (was bf16): halves quantization noise, same
    matmul/DVE throughput.
  - exp() split between ScalarE (ACT spline exp) and VectorE: on 4 of
    16 j-iterations per steady block the DVE computes a Schraudolph
    exp -- one tensor_scalar op producing fp16 *bit patterns* via an
    int16 bitcast view (bits = round(s*A + B) makes 2^t piecewise-
    linear in the mantissa; bias c=-57.6 tuned for zero-mean ripple).
    Block 0 runs all-ACT (it is PE-bound, so ACT idles there anyway).
    Without the split the ACT engine is critical at ~167us.
  - Deep AV lag (h0 by 4 iters, h1 by 5): the exp stream (ACT 1081ns /
    DVE 1224ns per tile) exceeds per-iteration PE work, but fits on
    average; deep lag + 10 pt buffers absorb consecutive-ACT bursts.
  - Lean lead-in: xT DMA s-slice-major so Q^T/K^T chunk 0 start after
    1/4 of the data lands; only those two chunks precede block 0.
  - Lean tail: p0-halves of the last 4 output chunks staged during
    block 7; the last block's h3 output is projected straight out of
    the norm tmpb tile (vs a partition-0-aligned Wo copy) so no
    sbuf->sbuf oT DMA sits in the tail critical path; tail out-DMAs
    alternate the two HW DGE queues (each DIRECT2D dispatch costs
    ~650ns of sequencer time).
  - PSUM: st 2x[128,1024] + po 2x[65,512] + x 2x[128,512] = 8 banks
    (AV lag lets po run at bufs=2, freeing a bank to double-buffer the
    x slot used by V chunks / deferred QK / projections / norm bcasts).
  - Emission order per iter: scores -> extras -> exp -> AVs -> lazy
    norm; projections deferred to j>=5 so the oT h1-half sbuf DMA
    (issued at j=2) lands before the first projection reads it.

Steady state is PSUM-write-port bound (~136us of port time total);
V chunks 0-3 run in the lead as the HAM warm-up, two iterations are
carried across each block boundary, and the tail is two matmul waves
overlapping the final normalization. Timings are chip-P-state
dependent: ~174us in the fast state, ~207us when the part downclocks
(~1.18x on every engine).
"""

import os
import sys

for _p in ("/opt/trn_rl_repo",):
    if os.path.isdir(_p) and _p not in sys.path:
        sys.path.append(_p)

import numpy as np

import concourse.bass as bass
import concourse.tile as tile
from concourse import bacc, mybir
from concourse.bass_utils import run_bass_kernel_spmd

F16 = mybir.dt.float16
F32 = mybir.dt.float32
I16 = mybir.dt.int16

B, S, DM = 4, 2048, 512
H, DH = 8, 64
HPC = 4  # heads per core
DQ = HPC * DH  # 256: per-core slice of the inner dim
N_CORES = 8
SCALE = DH**-0.5

# Schraudolph exp-as-bitcast constants (fp16 target):
#   exp(s*SCALE) = 2^(s*SCALE*log2e); fp16 bits ~= 1024*t + 15360 + c
# c tuned so the piecewise-linear mantissa ripple is zero-mean over the
# observed t distribution (minimizes net softmax error).
LOG2E = 1.4426950408889634
SCH_A = SCALE * LOG2E * 1024.0
SCH_C = -57.6
SCH_B = 15360.0 + SCH_C  # round-to-nearest convert assumed

# j-iterations whose exp runs on the DVE (per 16-iter block)
DVE_EXP_STEADY = frozenset((4, 7, 10, 13))
DVE_EXP_BLOCK0 = frozenset()  # block 0 is PE-bound; ACT has slack

AF = mybir.ActivationFunctionType
ALU = mybir.AluOpType

# exported for test harnesses
LAST_EXEC_TIME_NS = None
LAST_RESULT = None

_CACHED_NC = None


def _kernel_body(tc, xT_d, wq_d, wk_d, wv_d, wo_d, out_d):
    from contextlib import ExitStack

    nc = tc.nc
    with ExitStack() as ctx:
        consts = ctx.enter_context(tc.tile_pool(name="consts", bufs=1))
        ptp = ctx.enter_context(tc.tile_pool(name="pt", bufs=10))
        normp = ctx.enter_context(tc.tile_pool(name="norm", bufs=3))
        foutp = ctx.enter_context(tc.tile_pool(name="fout", bufs=4))
        # PSUM budget (8 banks): "s" 2x[128,1024]=4, "o" 2x[65,512]=2, "x" 2
        # (AV lag 4/5 means a block's po tiles are first written at j=4/5,
        # after the previous block's lazy norm frees its slots, so bufs=2
        # suffices and the x slot gets double-buffered instead)
        ps_s = ctx.enter_context(tc.tile_pool(name="ps_s", bufs=2, space="PSUM"))
        ps_o = ctx.enter_context(tc.tile_pool(name="ps_o", bufs=2, space="PSUM"))
        ps_x = ctx.enter_context(tc.tile_pool(name="ps_x", bufs=2, space="PSUM"))

        sb_xT = consts.tile([128, 4, S], F16)  # X^T: k-chunk c -> [:, c, :]
        sb_wq = consts.tile([128, 4, DQ], F16)
        sb_wk = consts.tile([128, 4, DQ], F16)
        sb_wv = consts.tile([128, 4, DQ], F16)
        sb_wo = consts.tile([128, 2, DM], F16)  # d'-chunk p -> [:, p, :]
        sb_wo2 = consts.tile([64, DM], F16)  # Wo rows of head 3, partition-0 based
        sb_qT = consts.tile([128, 2, S], F16)  # dq-chunk (head pair) p
        sb_kT = consts.tile([128, 2, S], F16)
        sb_v = consts.tile([128, 16, HPC, 66], F16)  # V_aug; col 64 = ones
        sb_oT = consts.tile([128, 2, S], F16)  # normalized O^T
        sb_warm = consts.tile([128, 512], F16)  # PE warmup fodder
        sb_one = consts.tile([128, 64], F16)  # all-ones (bcast stationary)

        nc.vector.memset(sb_one[:], 1.0)
        nc.vector.memset(sb_v[:, :, :, 64:66], 1.0)
        nc.vector.memset(sb_warm[:], 1.0)
        # DMA order: wq/wk first (lead QK chunks need them), then the first
        # s-slice of every xT k-chunk (lead QK chunk 0 contracts over all 4
        # k-chunks but only s-cols 0:512), then wv, then the rest of xT,
        # then wo (first needed ~90us in).
        nc.sync.dma_start(sb_wq[:], wq_d.rearrange("(c p) d -> p c d", p=128))
        xT_r = xT_d.rearrange("(c p) s -> c p s", p=128)
        for kc in range(4):
            nc.sync.dma_start(sb_xT[:, kc, 0:512], xT_r[kc, :, 0:512])
        nc.sync.dma_start(sb_wk[:], wk_d.rearrange("(c p) d -> p c d", p=128))
        nc.sync.dma_start(sb_wv[:], wv_d.rearrange("(c p) d -> p c d", p=128))
        for sc in range(1, 4):
            for kc in range(4):
                nc.sync.dma_start(
                    sb_xT[:, kc, sc * 512 : (sc + 1) * 512],
                    xT_r[kc, :, sc * 512 : (sc + 1) * 512],
                )
        nc.sync.dma_start(sb_wo[:], wo_d.rearrange("(c p) d -> p c d", p=128))
        nc.sync.dma_start(sb_wo2[:], wo_d[192:256, :])

        # Preload the exp table while the first xT slice is in flight (one
        # dummy matmul feeds the warm-up activation); the PE HAM warm-up is
        # real work now -- V chunks 0-3 computed in the lead below.
        pw = ps_x.tile([128, 512], F32, tag="x")
        nc.tensor.matmul(
            pw[:], lhsT=sb_warm[:, 0:128], rhs=sb_warm[:], start=True, stop=True
        )
        warm_act = normp.tile([1, 4], F32, tag="wact")
        nc.scalar.activation(warm_act[:], pw[0:1, 0:4], AF.Exp, scale=-1.0)

        def evac_qk(dst_sb, p, isl, pq):
            nc.vector.tensor_copy(dst_sb[:, p, isl], pq[:])

        def emit_qk_chunk(w_sb, dst_sb, p, c, pool_tag=("ps_s", "s")):
            """One [128,512] chunk of Q^T or K^T for head-pair p."""
            isl = slice(c * 512, (c + 1) * 512)
            pool = {"ps_s": ps_s, "ps_o": ps_o, "ps_x": ps_x}[pool_tag[0]]
            pq = pool.tile([128, 512], F32, tag=pool_tag[1], name="pqk")
            for kc in range(4):
                nc.tensor.matmul(
                    pq[:],
                    lhsT=w_sb[:, kc, p * 128 : (p + 1) * 128],
                    rhs=sb_xT[:, kc, isl],
                    start=(kc == 0),
                    stop=(kc == 3),
                )
            evac_qk(dst_sb, p, isl, pq)

        def emit_qk_chunk_mm(w_sb, p, c, kc, pq):
            nc.tensor.matmul(
                pq[:],
                lhsT=w_sb[:, kc, p * 128 : (p + 1) * 128],
                rhs=sb_xT[:, kc, c * 512 : (c + 1) * 512],
                start=(kc == 0),
                stop=(kc == 3),
            )

        def emit_v_chunk(sc):
            """V natural [s,dv] for s-chunk sc (all 4 heads)."""
            pv = ps_x.tile([128, DQ], F32, tag="x", name="pv")
            for kc in range(4):
                nc.tensor.matmul(
                    pv[:],
                    lhsT=sb_xT[:, kc, sc * 128 : (sc + 1) * 128],
                    rhs=sb_wv[:, kc, :],
                    start=(kc == 0),
                    stop=(kc == 3),
                )
            nc.vector.tensor_copy(
                sb_v[:, sc, :, 0:64], pv.rearrange("p (h d) -> p h d", h=HPC)
            )

        # deferred single-MM work queues, stepped one MM per j-iter
        pending_qk = []
        qk_state = {"chunk": None, "tile": None, "kc": 0}

        def step_pending_qk():
            stt = qk_state
            if stt["chunk"] is None:
                if not pending_qk:
                    return False
                stt["chunk"] = pending_qk.pop(0)
                stt["tile"] = ps_x.tile([128, 512], F32, tag="x", name="pqk1")
                stt["kc"] = 0
            w_sb, dst_sb, p, c = stt["chunk"]
            emit_qk_chunk_mm(w_sb, p, c, stt["kc"], stt["tile"])
            stt["kc"] += 1
            if stt["kc"] == 4:
                evac_qk(dst_sb, p, slice(c * 512, (c + 1) * 512), stt["tile"])
                stt["chunk"] = None
            return True

        # tail staging: p0-halves of the last four chunks precomputed during
        # block 7 so the tail needs only one matmul + DVE add per chunk
        sb_stage = consts.tile([128, 4, 512], F32)
        pending_proj = []
        pending_stage = []
        proj_state = {"c2": None, "tile": None, "p": 0}

        def step_pending_proj():
            stt = proj_state
            if stt["c2"] is None:
                if not pending_proj:
                    if pending_stage:
                        c2 = pending_stage.pop(0)
                        pf0 = ps_x.tile([128, 512], F32, tag="x", name="pf0")
                        nc.tensor.matmul(
                            pf0[:],
                            lhsT=sb_oT[:, 0, c2 * 128 : (c2 + 1) * 128],
                            rhs=sb_wo[:, 0, :],
                            start=True,
                            stop=True,
                        )
                        nc.vector.tensor_copy(sb_stage[:, c2 - 12, :], pf0[:])
                        return True
                    return False
                stt["c2"] = pending_proj.pop(0)
                stt["tile"] = ps_x.tile([128, 512], F32, tag="x", name="pf")
                stt["p"] = 0
            c2, p = stt["c2"], stt["p"]
            nc.tensor.matmul(
                stt["tile"][:],
                lhsT=sb_oT[:, p, c2 * 128 : (c2 + 1) * 128],
                rhs=sb_wo[:, p, :],
                start=(p == 0),
                stop=(p == 1),
            )
            stt["p"] += 1
            if stt["p"] == 2:
                fo = foutp.tile([128, 512], F32, tag="fo")
                nc.vector.tensor_copy(fo[:], stt["tile"][:])
                nc.sync.dma_start(out_d[c2 * 128 : (c2 + 1) * 128, :], fo[:])
                stt["c2"] = None
            return True

        # ---- normalization of a finished block, 3 lazy steps ----
        def make_norm_steps(p, ic, po, tail=False):
            isl = slice(ic * 512, (ic + 1) * 512)
            held = {}

            def step_sums():
                for hi in range(2):
                    s = normp.tile([65, 512], F16, tag="sums", name=f"sums{hi}")
                    if tail and hi == 1:
                        # ACT is idle at the tail: run the second sums copy
                        # there so both proceed in parallel
                        nc.scalar.copy(s[64:65, :], po[hi][64:65, :])
                    else:
                        nc.vector.tensor_copy(s[64:65, :], po[hi][64:65, :])
                    held[hi] = s

            def step_head(hi):
                # in the tail both broadcasts issue back-to-back from
                # different PSUM pools (the single x slot would serialize)
                pool, tag = (ps_o, "o") if (tail and hi == 1) else (ps_x, "x")
                pb = pool.tile([64, 512], F32, tag=tag, name=f"pb{hi}")
                nc.tensor.matmul(
                    pb[:],
                    lhsT=sb_one[64:65, :],
                    rhs=held[hi][64:65, :],
                    start=True,
                    stop=True,
                )
                rec = normp.tile([64, 512], F32, tag="rec", name=f"rec{hi}")
                nc.vector.reciprocal_approx_fast(rec[:], pb[:])
                if hi == 0:
                    nc.vector.tensor_mul(sb_oT[0:64, p, isl], po[0][0:64, :], rec[:])
                else:
                    tmpb = normp.tile([64, 512], F16, tag="tmpb")
                    nc.vector.tensor_mul(tmpb[:], po[1][0:64, :], rec[:])
                    if tail:
                        held["tmpb"] = tmpb  # tail projs read it directly
                    else:
                        nc.sync.dma_start(sb_oT[64:128, p, isl], tmpb[:])

            return [step_sums, lambda: step_head(0), lambda: step_head(1)], held

        def emit_scores(p, ic, j):
            isl = slice(ic * 512, (ic + 1) * 512)
            jsl = slice(j * 128, (j + 1) * 128)
            st = ps_s.tile([128, 1024], F32, tag="s")
            nc.tensor.matmul(
                st[:, 0:512],
                lhsT=sb_kT[0:64, p, jsl],
                rhs=sb_qT[0:64, p, isl],
                start=True,
                stop=True,
            )
            nc.tensor.matmul(
                st[:, 512:1024],
                lhsT=sb_kT[64:128, p, jsl],
                rhs=sb_qT[64:128, p, isl],
                start=True,
                stop=True,
            )
            return st

        def emit_exp(st, on_dve):
            pt = ptp.tile([128, 1024], F16, tag="pt")
            if on_dve:
                # Schraudolph: fp16 bits = round(s*A + B), via int16 view
                nc.vector.tensor_scalar(
                    pt[:].bitcast(I16), st[:], SCH_A, SCH_B, ALU.mult, ALU.add
                )
            else:
                nc.scalar.activation(pt[:], st[:], AF.Exp, scale=SCALE)
            return pt

        # ---- lead: Q^T/K^T chunk 0 for pair 0 + V chunks 0-3 (all only
        # need the first xT s-slice; doubles as the PE HAM warm-up and
        # decongests block 0's first iterations) ----
        emit_qk_chunk(sb_wq, sb_qT, 0, 0, ("ps_s", "s"))
        emit_qk_chunk(sb_wk, sb_kT, 0, 0, ("ps_o", "o"))
        for sc in range(4):
            emit_v_chunk(sc)

        # deferred: Q^T pair-0 chunks 1-3 run inside block 0; pair-1 K^T/Q^T
        # run across the remaining p0 blocks
        pending_qk0 = [(sb_wq, sb_qT, 0, c) for c in range(1, 4)]
        for c in range(4):
            pending_qk.append((sb_wk, sb_kT, 1, c))
        for c in range(4):
            pending_qk.append((sb_wq, sb_qT, 1, c))

        pending_norm = []
        blocks = [(p, ic) for p in range(2) for ic in range(4)]

        def emit_av(po, p, hi, jj, ptt):
            nc.tensor.matmul(
                po[hi][:],
                lhsT=sb_v[:, jj, 2 * p + hi, 0:65],
                rhs=ptt[:, hi * 512 : (hi + 1) * 512],
                start=(jj == 0),
                stop=(jj == 15),
                skip_group_check=True,
            )

        carry_pts = {}
        for bi, (p, ic) in enumerate(blocks):
            block0 = bi == 0
            dve_set = DVE_EXP_BLOCK0 if block0 else DVE_EXP_STEADY
            po = [
                ps_o.tile([65, 512], F32, tag="o", name=f"po{hi}") for hi in range(2)
            ]
            if p == 1 and ic > 0:
                # previous ic's projection slice; its oT inputs complete
                # during this block's first two iterations (lazy norm)
                pending_proj.extend(range(4 * (ic - 1), 4 * ic))
                if ic == 3:
                    # stage p0-halves of the tail chunks in the free slots
                    pending_stage.extend(range(12, 16))
            pts = []  # pt tile per j (consumed by lagged AVs)
            for j in range(16):
                used_carry = j in carry_pts
                if used_carry:
                    pt = carry_pts.pop(j)  # scores+exp ran in previous block
                else:
                    st = emit_scores(p, ic, j)
                # extras: deferred matmuls keep PE fed; the x-slot is needed
                # by the norm broadcasts at j=1,2 so extras wait till j>=3
                if block0:
                    if 0 < j < 4:
                        emit_qk_chunk(sb_wk, sb_kT, 0, j, ("ps_o", "o"))
                    if j == 0:
                        emit_v_chunk(4)
                        emit_v_chunk(5)
                    elif j + 5 <= 15:
                        emit_v_chunk(j + 5)
                    if j >= 4 and pending_qk0:
                        w_sb, dst_sb, pp, c = pending_qk0[0]
                        stt = qk_state
                        if stt["chunk"] is None:
                            stt["chunk"] = pending_qk0.pop(0)
                            stt["tile"] = ps_x.tile(
                                [128, 512], F32, tag="x", name="pqk1"
                            )
                            stt["kc"] = 0
                        w_sb, dst_sb, pp, c = stt["chunk"]
                        emit_qk_chunk_mm(w_sb, pp, c, stt["kc"], stt["tile"])
                        stt["kc"] += 1
                        if stt["kc"] == 4:
                            evac_qk(
                                dst_sb, pp, slice(c * 512, (c + 1) * 512), stt["tile"]
                            )
                            stt["chunk"] = None
                elif j >= 3:
                    if p == 0:
                        step_pending_qk()
                    elif j >= 5:
                        # proj waits j>=5: chunk ic-1's oT h1 half arrives by
                        # sbuf DMA issued at j=2; give it slack before the
                        # first projection matmul reads it
                        step_pending_proj()
                if not used_carry:
                    pt = emit_exp(st, j in dve_set)

                # uniform AV lag (h0 by 4 iters, h1 by 5); AVs emitted before
                # the lazy-norm broadcasts so a norm waiting on the DVE sums
                # copy never blocks ready AVs in the in-order PE queue
                pts.append(pt)
                if j >= 4:
                    emit_av(po, p, 0, j - 4, pts[j - 4])
                if j >= 5:
                    emit_av(po, p, 1, j - 5, pts[j - 5])
                if pending_norm:
                    if j == 1:
                        pending_norm[0]()  # bcast+recip+mul head 0
                    elif j == 2:
                        pending_norm[1]()  # ... head 1
                        pending_norm = []
                if j == 15:
                    # flush AVs whose exp tiles are ready, then slot the
                    # carries (independent of exp 14/15) into the PE while
                    # those late exps finish, then flush the rest
                    for jj in (12, 13):
                        emit_av(po, p, 0, jj, pts[jj])
                    for jj in (11, 12, 13):
                        emit_av(po, p, 1, jj, pts[jj])
                    if bi + 1 < len(blocks):
                        np_, nic = blocks[bi + 1]
                        nset = DVE_EXP_BLOCK0 if bi + 1 == 0 else DVE_EXP_STEADY
                        for jn in (0, 1):
                            carry_pts[jn] = emit_exp(
                                emit_scores(np_, nic, jn), jn in nset
                            )
                    for jj in (14, 15):
                        emit_av(po, p, 0, jj, pts[jj])
                        emit_av(po, p, 1, jj, pts[jj])
                    pending_norm, norm_held = make_norm_steps(
                        p, ic, po, tail=(bi == 7)
                    )
                    pending_norm[0]()  # sums copies queue right behind AVs
                    pending_norm = pending_norm[1:]

        # ---- tail: last normalize + the four staged chunks ----
        for step in pending_norm:
            step()
        while pending_proj or proj_state["c2"] is not None or pending_stage:
            step_pending_proj()
        # each tail chunk: h2 half from oT (K=64) + h3 half straight from the
        # norm's tmpb tile (K=64, vs a partition-0-aligned Wo copy) -- no
        # sbuf->sbuf oT DMA in the chain -- then DVE add against the staged
        # p0 half. One out-DMA per chunk, alternating the two HW DGE queues
        # (each DIRECT2D dispatch costs ~650ns of sequencer time).
        # all four h2-half matmuls issue back-to-back (2 ps_s slots + 2
        # freed ps_o slots) and overlap the DVE finishing h3's recip/mul;
        # the mul1-gated h3-half matmuls then accumulate in a second wave
        tmpb = norm_held["tmpb"]
        pfs = []
        for n, c2 in enumerate(range(12, 16)):
            csl = slice(c2 * 128, (c2 + 1) * 128)
            pool, tag = (ps_s, "s") if n < 2 else (ps_o, "o")
            pf = pool.tile([128, 512], F32, tag=tag, name=f"pfz{n % 2}")
            nc.tensor.matmul(
                pf[:],
                lhsT=sb_oT[0:64, 1, csl],
                rhs=sb_wo[0:64, 1, :],
                start=True,
                stop=False,
            )
            pfs.append((c2, csl, pf))
        for n, (c2, csl, pf) in enumerate(pfs):
            nc.tensor.matmul(
                pf[:],
                lhsT=tmpb[:, (c2 - 12) * 128 : (c2 - 11) * 128],
                rhs=sb_wo2[:],
                start=False,
                stop=True,
            )
            fo = foutp.tile([128, 512], F32, tag="fo")
            nc.vector.tensor_add(fo[:], pf[:], sb_stage[:, c2 - 12, :])
            eng = nc.sync if n % 2 == 0 else nc.scalar
            eng.dma_start(out_d[csl, :], fo[:])


def _build():
    nc = bacc.Bacc("TRN2", target_bir_lowering=False, debug=False, num_devices=N_CORES)
    xT = nc.dram_tensor("xT", [DM, S], F16, kind="ExternalInput")
    wq = nc.dram_tensor("wq", [DM, DQ], F16, kind="ExternalInput")
    wk = nc.dram_tensor("wk", [DM, DQ], F16, kind="ExternalInput")
    wv = nc.dram_tensor("wv", [DM, DQ], F16, kind="ExternalInput")
    wo = nc.dram_tensor("wo", [DQ, DM], F16, kind="ExternalInput")
    out = nc.dram_tensor("out", [S, DM], F32, kind="ExternalOutput")
    with tile.TileContext(nc) as tc:
        _kernel_body(tc, xT.ap(), wq.ap(), wk.ap(), wv.ap(), wo.ap(), out.ap())
    nc.compile()
    return nc


def get_nc():
    global _CACHED_NC
    if _CACHED_NC is None:
        _CACHED_NC = _build()
    return _CACHED_NC


def _in_maps(hidden_states, Wq, Wk, Wv, Wo):
    f16 = np.float16
    maps = []
    for c in range(N_CORES):
        b, g = c // 2, c % 2
        cols = slice(g * DQ, (g + 1) * DQ)
        maps.append(
            {
                "xT": np.ascontiguousarray(hidden_states[b].T).astype(f16),
                "wq": np.ascontiguousarray(Wq[:, cols]).astype(f16),
                "wk": np.ascontiguousarray(Wk[:, cols]).astype(f16),
                "wv": np.ascontiguousarray(Wv[:, cols]).astype(f16),
                "wo": np.ascontiguousarray(Wo[cols, :]).astype(f16),
            }
        )
    return maps


def _ensure_profile_support():
    """Best-effort: register the axon NTFF profiling hook + defang the
    bucket upload (zero-egress container)."""
    import types

    try:
        import antenv

        if "antenv.axon_hooks" not in sys.modules:
            mod = types.ModuleType("antenv.axon_hooks")
            _h = {"hook": None}
            mod.set_axon_ntff_profile_hook = lambda h: _h.__setitem__("hook", h)
            mod.get_axon_ntff_profile_hook = lambda: _h["hook"]
            sys.modules["antenv.axon_hooks"] = mod
            antenv.axon_hooks = mod
        import antenv.axon_hooks as ah

        if ah.get_axon_ntff_profile_hook() is None:
            if "/root/.axon_site" not in sys.path:
                sys.path.append("/root/.axon_site")
            from trn_agent_boot.trn_boot import _ntff_profile_via_ctypes

            hook = _ntff_profile_via_ctypes("/opt/axon/libaxon_pjrt.so")
            if hook is not None:
                ah.set_axon_ntff_profile_hook(hook)
    except Exception:
        pass
    try:
        import concourse.bass_utils as bu

        bu.upload_artifacts = lambda tmpdir: tmpdir
    except Exception:
        pass


def kernel(hidden_states, Wq, Wk, Wv, Wo):
    global LAST_EXEC_TIME_NS, LAST_RESULT
    hidden_states = np.asarray(hidden_states, dtype=np.float32)
    Wq, Wk, Wv, Wo = (np.asarray(w, dtype=np.float32) for w in (Wq, Wk, Wv, Wo))

    trace = bool(os.environ.get("BASS_TRACE"))
    if trace:
        _ensure_profile_support()
    nc = get_nc()
    maps = _in_maps(hidden_states, Wq, Wk, Wv, Wo)
    res = run_bass_kernel_spmd(
        nc,
        maps,
        core_ids=list(range(N_CORES)),
        trace=trace,
        tmpdir=os.environ.get("BASS_TRACE_DIR") or None,
    )
    LAST_RESULT = res
    LAST_EXEC_TIME_NS = res.exec_time_ns

    out = np.empty((B, S, DM), dtype=np.float32)
    for b in range(B):
        out[b] = res.results[2 * b]["out"] + res.results[2 * b + 1]["out"]
    return out


if __name__ == "__main__":
    rng = np.random.default_rng(0)
    hs = rng.standard_normal((B, S, DM), dtype=np.float32)
    ws = [
        (rng.standard_normal((DM, DM), dtype=np.float32) / np.sqrt(DM))
        for _ in range(4)
    ]
    o = kernel(hs, *ws)
    print("out", o.shape, o.dtype, float(np.abs(o).mean()))
    print("exec_time_ns", LAST_EXEC_TIME_NS)


# revision 42
# speedup vs baseline: 1.0336x; 1.0336x over previous
"""Multi-head attention (B=4, S=2048, H=8, Dh=64, Dm=512) on 8 TRN2 NeuronCores.

Sharding: batch*head parallel. Core c owns batch b = c//2 and head group
g = c%2 (4 heads each). Each core computes QKV projection for its head
group, transposed-scores flash-style attention (no max subtraction --
scores ~ N(0,1) after 1/sqrt(Dh) scaling), and its partial output
projection against its 256 rows of Wo. The host sums the two partial
projections per batch.

Optimizations vs the first working kernel (194-230us -> ~174us):
  - Whole datapath in fp16 (was bf16): halves quantization noise, same
    matmul/DVE throughput.
  - exp() split between ScalarE (ACT spline exp) and VectorE: on 4 of
    16 j-iterations per steady block the DVE computes a Schraudolph
    exp -- one tensor_scalar op producing fp16 *bit patterns* via an
    int16 bitcast view (bits = round(s*A + B) makes 2^t piecewise-
    linear in the mantissa; bias c=-57.6 tuned for zero-mean ripple).
    Block 0 runs all-ACT (it is PE-bound, so ACT idles there anyway).
    Without the split the ACT engine is critical at ~167us.
  - Deep AV lag (h0 by 4 iters, h1 by 5): the exp stream (ACT 1081ns /
    DVE 1224ns per tile) exceeds per-iteration PE work, but fits on
    average; deep lag + 10 pt buffers absorb consecutive-ACT bursts.
  - Lean lead-in: xT DMA s-slice-major so Q^T/K^T chunk 0 start after
    1/4 of the data lands; only those two chunks precede block 0.
  - Lean tail: p0-halves of the last 4 output chunks staged during
    block 7; the last block's h3 output is projected straight out of
    the norm tmpb tile (vs a partition-0-aligned Wo copy) so no
    sbuf->sbuf oT DMA sits in the tail critical path; tail out-DMAs
    alternate the two HW DGE queues (each DIRECT2D dispatch costs
    ~650ns of sequencer time).
  - PSUM: st 2x[128,1024] + po 2x[65,512] + x 2x[128,512] = 8 banks
    (AV lag lets po run at bufs=2, freeing a bank to double-buffer the
    x slot used by V chunks / deferred QK / projections / norm bcasts).
  - Emission order per iter: scores -> extras -> exp -> AVs -> lazy
    norm; projections deferred to j>=5 so the oT h1-half sbuf DMA
    (issued at j=2) lands before the first projection reads it.

Steady state is PSUM-write-port bound (~136us of port time total);
V chunks 0-3 run in the lead as the HAM warm-up, two iterations are
carried across each block boundary, and the tail is two matmul waves
overlapping the final normalization. Timings are chip-P-state
dependent: ~174us in the fast state, ~207us when the part downclocks
(~1.18x on every engine).
"""

import os
import sys

for _p in ("/opt/trn_rl_repo",):
    if os.path.isdir(_p) and _p not in sys.path:
        sys.path.append(_p)

import numpy as np

import concourse.bass as bass
import concourse.tile as tile
from concourse import bacc, mybir
from concourse.bass_utils import run_bass_kernel_spmd

F16 = mybir.dt.float16
F32 = mybir.dt.float32
I16 = mybir.dt.int16

B, S, DM = 4, 2048, 512
H, DH = 8, 64
HPC = 4  # heads per core
DQ = HPC * DH  # 256: per-core slice of the inner dim
N_CORES = 8
SCALE = DH**-0.5

# Schraudolph exp-as-bitcast constants (fp16 target):
#   exp(s*SCALE) = 2^(s*SCALE*log2e); fp16 bits ~= 1024*t + 15360 + c
# c tuned so the piecewise-linear mantissa ripple is zero-mean over the
# observed t distribution (minimizes net softmax error).
LOG2E = 1.4426950408889634
SCH_A = SCALE * LOG2E * 1024.0
SCH_C = -57.6
SCH_B = 15360.0 + SCH_C  # round-to-nearest convert assumed

# j-iterations whose exp runs on the DVE (per 16-iter block)
DVE_EXP_STEADY = frozenset((4, 7, 10, 13))
DVE_EXP_BLOCK0 = frozenset()  # block 0 is PE-bound; ACT has slack

AF = mybir.ActivationFunctionType
ALU = mybir.AluOpType

# exported for test harnesses
LAST_EXEC_TIME_NS = None
LAST_RESULT = None

_CACHED_NC = None


def _kernel_body(tc, xT_d, wq_d, wk_d, wv_d, wo_d, out_d):
    from contextlib import ExitStack

    nc = tc.nc
    with ExitStack() as ctx:
        consts = ctx.enter_context(tc.tile_pool(name="consts", bufs=1))
        ptp = ctx.enter_context(tc.tile_pool(name="pt", bufs=10))
        normp = ctx.enter_context(tc.tile_pool(name="norm", bufs=3))
        foutp = ctx.enter_context(tc.tile_pool(name="fout", bufs=4))
        # PSUM budget (8 banks): "s" 2x[128,1024]=4, "o" 2x[65,512]=2, "x" 2
        # (AV lag 4/5 means a block's po tiles are first written at j=4/5,
        # after the previous block's lazy norm frees its slots, so bufs=2
        # suffices and the x slot gets double-buffered instead)
        ps_s = ctx.enter_context(tc.tile_pool(name="ps_s", bufs=2, space="PSUM"))
        ps_o = ctx.enter_context(tc.tile_pool(name="ps_o", bufs=2, space="PSUM"))
        ps_x = ctx.enter_context(tc.tile_pool(name="ps_x", bufs=2, space="PSUM"))

        sb_xT = consts.tile([128, 4, S], F16)  # X^T: k-chunk c -> [:, c, :]
        sb_wq = consts.tile([128, 4, DQ], F16)
        sb_wk = consts.tile([128, 4, DQ], F16)
        sb_wv = consts.tile([128, 4, DQ], F16)
        sb_wo = consts.tile([128, 2, DM], F16)  # d'-chunk p -> [:, p, :]
        sb_wo2 = consts.tile([64, DM], F16)  # Wo rows of head 3, partition-0 based
        sb_qT = consts.tile([128, 2, S], F16)  # dq-chunk (head pair) p
        sb_kT = consts.tile([128, 2, S], F16)
        sb_v = consts.tile([128, 16, HPC, 66], F16)  # V_aug; col 64 = ones
        sb_oT = consts.tile([128, 2, S], F16)  # normalized O^T
        sb_warm = consts.tile([128, 512], F16)  # PE warmup fodder
        sb_one = consts.tile([128, 64], F16)  # all-ones (bcast stationary)

        nc.vector.memset(sb_one[:], 1.0)
        nc.vector.memset(sb_v[:, :, :, 64:66], 1.0)
        nc.vector.memset(sb_warm[:], 1.0)
        # DMA order: wq/wk first (lead QK chunks need them), then the first
        # s-slice of every xT k-chunk (lead QK chunk 0 contracts over all 4
        # k-chunks but only s-cols 0:512), then wv, then the rest of xT,
        # then wo (first needed ~90us in).
        nc.sync.dma_start(sb_wq[:], wq_d.rearrange("(c p) d -> p c d", p=128))
        xT_r = xT_d.rearrange("(c p) s -> c p s", p=128)
        for kc in range(4):
            nc.sync.dma_start(sb_xT[:, kc, 0:512], xT_r[kc, :, 0:512])
        nc.sync.dma_start(sb_wk[:], wk_d.rearrange("(c p) d -> p c d", p=128))
        nc.sync.dma_start(sb_wv[:], wv_d.rearrange("(c p) d -> p c d", p=128))
        for sc in range(1, 4):
            for kc in range(4):
                nc.sync.dma_start(
                    sb_xT[:, kc, sc * 512 : (sc + 1) * 512],
                    xT_r[kc, :, sc * 512 : (sc + 1) * 512],
                )
        nc.sync.dma_start(sb_wo[:], wo_d.rearrange("(c p) d -> p c d", p=128))
        nc.sync.dma_start(sb_wo2[:], wo_d[192:256, :])

        # Preload the exp table while the first xT slice is in flight (one
        # dummy matmul feeds the warm-up activation); the PE HAM warm-up is
        # real work now -- V chunks 0-3 computed in the lead below.
        pw = ps_x.tile([128, 512], F32, tag="x")
        nc.tensor.matmul(
            pw[:], lhsT=sb_warm[:, 0:128], rhs=sb_warm[:], start=True, stop=True
        )
        warm_act = normp.tile([1, 4], F32, tag="wact")
        nc.scalar.activation(warm_act[:], pw[0:1, 0:4], AF.Exp, scale=-1.0)

        def evac_qk(dst_sb, p, isl, pq):
            nc.vector.tensor_copy(dst_sb[:, p, isl], pq[:])

        def emit_qk_chunk(w_sb, dst_sb, p, c, pool_tag=("ps_s", "s")):
            """One [128,512] chunk of Q^T or K^T for head-pair p."""
            isl = slice(c * 512, (c + 1) * 512)
            pool = {"ps_s": ps_s, "ps_o": ps_o, "ps_x": ps_x}[pool_tag[0]]
            pq = pool.tile([128, 512], F32, tag=pool_tag[1], name="pqk")
            for kc in range(4):
                nc.tensor.matmul(
                    pq[:],
                    lhsT=w_sb[:, kc, p * 128 : (p + 1) * 128],
                    rhs=sb_xT[:, kc, isl],
                    start=(kc == 0),
                    stop=(kc == 3),
                )
            evac_qk(dst_sb, p, isl, pq)

        def emit_qk_chunk_mm(w_sb, p, c, kc, pq):
            nc.tensor.matmul(
                pq[:],
                lhsT=w_sb[:, kc, p * 128 : (p + 1) * 128],
                rhs=sb_xT[:, kc, c * 512 : (c + 1) * 512],
                start=(kc == 0),
                stop=(kc == 3),
            )

        def emit_v_chunk(sc):
            """V natural [s,dv] for s-chunk sc (all 4 heads)."""
            pv = ps_x.tile([128, DQ], F32, tag="x", name="pv")
            for kc in range(4):
                nc.tensor.matmul(
                    pv[:],
                    lhsT=sb_xT[:, kc, sc * 128 : (sc + 1) * 128],
                    rhs=sb_wv[:, kc, :],
                    start=(kc == 0),
                    stop=(kc == 3),
                )
            nc.vector.tensor_copy(
                sb_v[:, sc, :, 0:64], pv.rearrange("p (h d) -> p h d", h=HPC)
            )

        # deferred single-MM work queues, stepped one MM per j-iter
        pending_qk = []
        qk_state = {"chunk": None, "tile": None, "kc": 0}

        def step_pending_qk():
            stt = qk_state
            if stt["chunk"] is None:
                if not pending_qk:
                    return False
                stt["chunk"] = pending_qk.pop(0)
                stt["tile"] = ps_x.tile([128, 512], F32, tag="x", name="pqk1")
                stt["kc"] = 0
            w_sb, dst_sb, p, c = stt["chunk"]
            emit_qk_chunk_mm(w_sb, p, c, stt["kc"], stt["tile"])
            stt["kc"] += 1
            if stt["kc"] == 4:
                evac_qk(dst_sb, p, slice(c * 512, (c + 1) * 512), stt["tile"])
                stt["chunk"] = None
            return True

        # tail staging: p0-halves of the last four chunks precomputed during
        # block 7 so the tail needs only one matmul + DVE add per chunk
        sb_stage = consts.tile([128, 4, 512], F32)
        pending_proj = []
        pending_stage = []
        proj_state = {"c2": None, "tile": None, "p": 0}

        def step_pending_proj():
            stt = proj_state
            if stt["c2"] is None:
                if not pending_proj:
                    if pending_stage:
                        c2 = pending_stage.pop(0)
                        pf0 = ps_x.tile([128, 512], F32, tag="x", name="pf0")
                        nc.tensor.matmul(
                            pf0[:],
                            lhsT=sb_oT[:, 0, c2 * 128 : (c2 + 1) * 128],
                            rhs=sb_wo[:, 0, :],
                            start=True,
                            stop=True,
                        )
                        nc.vector.tensor_copy(sb_stage[:, c2 - 12, :], pf0[:])
                        return True
                    return False
                stt["c2"] = pending_proj.pop(0)
                stt["tile"] = ps_x.tile([128, 512], F32, tag="x", name="pf")
                stt["p"] = 0
            c2, p = stt["c2"], stt["p"]
            nc.tensor.matmul(
                stt["tile"][:],
                lhsT=sb_oT[:, p, c2 * 128 : (c2 + 1) * 128],
                rhs=sb_wo[:, p, :],
                start=(p == 0),
                stop=(p == 1),
            )
            stt["p"] += 1
            if stt["p"] == 2:
                fo = foutp.tile([128, 512], F32, tag="fo")
                nc.vector.tensor_copy(fo[:], stt["tile"][:])
                nc.sync.dma_start(out_d[c2 * 128 : (c2 + 1) * 128, :], fo[:])
                stt["c2"] = None
            return True

        # ---- normalization of a finished block, 3 lazy steps ----
        def make_norm_steps(p, ic, po, tail=False):
            isl = slice(ic * 512, (ic + 1) * 512)
            held = {}

            def step_sums():
                for hi in range(2):
                    s = normp.tile([65, 512], F16, tag="sums", name=f"sums{hi}")
                    if tail and hi == 1:
                        # ACT is idle at the tail: run the second sums copy
                        # there so both proceed in parallel
                        nc.scalar.copy(s[64:65, :], po[hi][64:65, :])
                    else:
                        nc.vector.tensor_copy(s[64:65, :], po[hi][64:65, :])
                    held[hi] = s

            def step_head(hi):
                # in the tail both broadcasts issue back-to-back from
                # different PSUM pools (the single x slot would serialize)
                pool, tag = (ps_o, "o") if (tail and hi == 1) else (ps_x, "x")
                pb = pool.tile([64, 512], F32, tag=tag, name=f"pb{hi}")
                nc.tensor.matmul(
                    pb[:],
                    lhsT=sb_one[64:65, :],
                    rhs=held[hi][64:65, :],
                    start=True,
                    stop=True,
                )
                rec = normp.tile([64, 512], F32, tag="rec", name=f"rec{hi}")
                nc.vector.reciprocal_approx_fast(rec[:], pb[:])
                if hi == 0:
                    nc.vector.tensor_mul(sb_oT[0:64, p, isl], po[0][0:64, :], rec[:])
                else:
                    tmpb = normp.tile([64, 512], F16, tag="tmpb")
                    nc.vector.tensor_mul(tmpb[:], po[1][0:64, :], rec[:])
                    if tail:
                        held["tmpb"] = tmpb  # tail projs read it directly
                    else:
                        nc.sync.dma_start(sb_oT[64:128, p, isl], tmpb[:])

            return [step_sums, lambda: step_head(0), lambda: step_head(1)], held

        def emit_scores(p, ic, j):
            isl = slice(ic * 512, (ic + 1) * 512)
            jsl = slice(j * 128, (j + 1) * 128)
            st = ps_s.tile([128, 1024], F32, tag="s")
            nc.tensor.matmul(
                st[:, 0:512],
                lhsT=sb_kT[0:64, p, jsl],
                rhs=sb_qT[0:64, p, isl],
                start=True,
                stop=True,
            )
            nc.tensor.matmul(
                st[:, 512:1024],
                lhsT=sb_kT[64:128, p, jsl],
                rhs=sb_qT[64:128, p, isl],
                start=True,
                stop=True,
            )
            return st

        def emit_exp(st, on_dve):
            pt = ptp.tile([128, 1024], F16, tag="pt")
            if on_dve:
                # Schraudolph: fp16 bits = round(s*A + B), via int16 view
                nc.vector.tensor_scalar(
                    pt[:].bitcast(I16), st[:], SCH_A, SCH_B, ALU.mult, ALU.add
                )
            else:
                nc.scalar.activation(pt[:], st[:], AF.Exp, scale=SCALE)
            return pt

        # ---- lead: Q^T/K^T chunk 0 for pair 0 + V chunks 0-3 (all only
        # need the first xT s-slice; doubles as the PE HAM warm-up and
        # decongests block 0's first iterations) ----
        emit_qk_chunk(sb_wq, sb_qT, 0, 0, ("ps_s", "s"))
        emit_qk_chunk(sb_wk, sb_kT, 0, 0, ("ps_o", "o"))
        for sc in range(4):
            emit_v_chunk(sc)

        # deferred: Q^T pair-0 chunks 1-3 run inside block 0; pair-1 K^T/Q^T
        # run across the remaining p0 blocks
        pending_qk0 = [(sb_wq, sb_qT, 0, c) for c in range(1, 4)]
        for c in range(4):
            pending_qk.append((sb_wk, sb_kT, 1, c))
        for c in range(4):
            pending_qk.append((sb_wq, sb_qT, 1, c))

        pending_norm = []
        blocks = [(p, ic) for p in range(2) for ic in range(4)]

        def emit_av(po, p, hi, jj, ptt):
            nc.tensor.matmul(
                po[hi][:],
                lhsT=sb_v[:, jj, 2 * p + hi, 0:65],
                rhs=ptt[:, hi * 512 : (hi + 1) * 512],
                start=(jj == 0),
                stop=(jj == 15),
                skip_group_check=True,
            )

        carry_pts = {}
        for bi, (p, ic) in enumerate(blocks):
            block0 = bi == 0
            dve_set = DVE_EXP_BLOCK0 if block0 else DVE_EXP_STEADY
            po = [
                ps_o.tile([65, 512], F32, tag="o", name=f"po{hi}") for hi in range(2)
            ]
            if p == 1 and ic > 0:
                # previous ic's projection slice; its oT inputs complete
                # during this block's first two iterations (lazy norm)
                pending_proj.extend(range(4 * (ic - 1), 4 * ic))
                if ic == 3:
                    # stage p0-halves of the tail chunks in the free slots
                    pending_stage.extend(range(12, 16))
            pts = []  # pt tile per j (consumed by lagged AVs)
            for j in range(16):
                used_carry = j in carry_pts
                if used_carry:
                    pt = carry_pts.pop(j)  # scores+exp ran in previous block
                else:
                    st = emit_scores(p, ic, j)
                # extras: deferred matmuls keep PE fed; the x-slot is needed
                # by the norm broadcasts at j=1,2 so extras wait till j>=3
                if block0:
                    if 0 < j < 4:
                        emit_qk_chunk(sb_wk, sb_kT, 0, j, ("ps_o", "o"))
                    if j == 0:
                        emit_v_chunk(4)
                        emit_v_chunk(5)
                    elif j + 5 <= 15:
                        emit_v_chunk(j + 5)
                    if j >= 4 and pending_qk0:
                        w_sb, dst_sb, pp, c = pending_qk0[0]
                        stt = qk_state
                        if stt["chunk"] is None:
                            stt["chunk"] = pending_qk0.pop(0)
                            stt["tile"] = ps_x.tile(
                                [128, 512], F32, tag="x", name="pqk1"
                            )
                            stt["kc"] = 0
                        w_sb, dst_sb, pp, c = stt["chunk"]
                        emit_qk_chunk_mm(w_sb, pp, c, stt["kc"], stt["tile"])
                        stt["kc"] += 1
                        if stt["kc"] == 4:
                            evac_qk(
                                dst_sb, pp, slice(c * 512, (c + 1) * 512), stt["tile"]
                            )
                            stt["chunk"] = None
                elif j >= 3:
                    if p == 0:
                        step_pending_qk()
                    elif j >= 5:
                        # proj waits j>=5: chunk ic-1's oT h1 half arrives by
                        # sbuf DMA issued at j=2; give it slack before the
                        # first projection matmul reads it
                        step_pending_proj()
                if not used_carry:
                    pt = emit_exp(st, j in dve_set)

                # uniform AV lag (h0 by 4 iters, h1 by 5); AVs emitted before
                # the lazy-norm broadcasts so a norm waiting on the DVE sums
                # copy never blocks ready AVs in the in-order PE queue
                pts.append(pt)
                if j >= 4:
                    emit_av(po, p, 0, j - 4, pts[j - 4])
                if j >= 5:
                    emit_av(po, p, 1, j - 5, pts[j - 5])
                if pending_norm:
                    if j == 1:
                        pending_norm[0]()  # bcast+recip+mul head 0
                    elif j == 2:
                        pending_norm[1]()  # ... head 1
                        pending_norm = []
                if j == 15:
                    for jj in (12, 13, 14, 15):
                        emit_av(po, p, 0, jj, pts[jj])
                    for jj in (11, 12, 13, 14, 15):
                        emit_av(po, p, 1, jj, pts[jj])
                    pending_norm, norm_held = make_norm_steps(
                        p, ic, po, tail=(bi == 7)
                    )
                    pending_norm[0]()  # sums copies queue right behind AVs
                    pending_norm = pending_norm[1:]
                    if bi + 1 < len(blocks):
                        # cross-block pipeline: next block's first TWO
                        # scores+exp, so ACT stays fed through the AV flush
                        np_, nic = blocks[bi + 1]
                        nset = DVE_EXP_BLOCK0 if bi + 1 == 0 else DVE_EXP_STEADY
                        for jn in (0, 1):
                            carry_pts[jn] = emit_exp(
                                emit_scores(np_, nic, jn), jn in nset
                            )

        # ---- tail: last normalize + the four staged chunks ----
        for step in pending_norm:
            step()
        while pending_proj or proj_state["c2"] is not None or pending_stage:
            step_pending_proj()
        # each tail chunk: h2 half from oT (K=64) + h3 half straight from the
        # norm's tmpb tile (K=64, vs a partition-0-aligned Wo copy) -- no
        # sbuf->sbuf oT DMA in the chain -- then DVE add against the staged
        # p0 half. One out-DMA per chunk, alternating the two HW DGE queues
        # (each DIRECT2D dispatch costs ~650ns of sequencer time).
        # all four h2-half matmuls issue back-to-back (2 ps_s slots + 2
        # freed ps_o slots) and overlap the DVE finishing h3's recip/mul;
        # the mul1-gated h3-half matmuls then accumulate in a second wave
        tmpb = norm_held["tmpb"]
        pfs = []
        for n, c2 in enumerate(range(12, 16)):
            csl = slice(c2 * 128, (c2 + 1) * 128)
            pool, tag = (ps_s, "s") if n < 2 else (ps_o, "o")
            pf = pool.tile([128, 512], F32, tag=tag, name=f"pfz{n % 2}")
            nc.tensor.matmul(
                pf[:],
                lhsT=sb_oT[0:64, 1, csl],
                rhs=sb_wo[0:64, 1, :],
                start=True,
                stop=False,
            )
            pfs.append((c2, csl, pf))
        for n, (c2, csl, pf) in enumerate(pfs):
            nc.tensor.matmul(
                pf[:],
                lhsT=tmpb[:, (c2 - 12) * 128 : (c2 - 11) * 128],
                rhs=sb_wo2[:],
                start=False,
                stop=True,
            )
            fo = foutp.tile([128, 512], F32, tag="fo")
            nc.vector.tensor_add(fo[:], pf[:], sb_stage[:, c2 - 12, :])
            eng = nc.sync if n % 2 == 0 else nc.scalar
            eng.dma_start(out_d[csl, :], fo[:])


def _build():
    nc = bacc.Bacc("TRN2", target_bir_lowering=False, debug=False, num_devices=N_CORES)
    xT = nc.dram_tensor("xT", [DM, S], F16, kind="ExternalInput")
    wq = nc.dram_tensor("wq", [DM, DQ], F16, kind="ExternalInput")
    wk = nc.dram_tensor("wk", [DM, DQ], F16, kind="ExternalInput")
    wv = nc.dram_tensor("wv", [DM, DQ], F16, kind="ExternalInput")
    wo = nc.dram_tensor("wo", [DQ, DM], F16, kind="ExternalInput")
    out = nc.dram_tensor("out", [S, DM], F32, kind="ExternalOutput")
    with tile.TileContext(nc) as tc:
        _kernel_body(tc, xT.ap(), wq.ap(), wk.ap(), wv.ap(), wo.ap(), out.ap())
    nc.compile()
    return nc


def get_nc():
    global _CACHED_NC
    if _CACHED_NC is None:
        _CACHED_NC = _build()
    return _CACHED_NC


def _in_maps(hidden_states, Wq, Wk, Wv, Wo):
    f16 = np.float16
    maps = []
    for c in range(N_CORES):
        b, g = c // 2, c % 2
        cols = slice(g * DQ, (g + 1) * DQ)
        maps.append(
            {
                "xT": np.ascontiguousarray(hidden_states[b].T).astype(f16),
                "wq": np.ascontiguousarray(Wq[:, cols]).astype(f16),
                "wk": np.ascontiguousarray(Wk[:, cols]).astype(f16),
                "wv": np.ascontiguousarray(Wv[:, cols]).astype(f16),
                "wo": np.ascontiguousarray(Wo[cols, :]).astype(f16),
            }
        )
    return maps


def _ensure_profile_support():
    """Best-effort: register the axon NTFF profiling hook + defang the
    bucket upload (zero-egress container)."""
    import types

    try:
        import antenv

        if "antenv.axon_hooks" not in sys.modules:
            mod = types.ModuleType("antenv.axon_hooks")
            _h = {"hook": None}
            mod.set_axon_ntff_profile_hook = lambda h: _h.__setitem__("hook", h)
            mod.get_axon_ntff_profile_hook = lambda: _h["hook"]
            sys.modules["antenv.axon_hooks"] = mod
            antenv.axon_hooks = mod
        import antenv.axon_hooks as ah

        if ah.get_axon_ntff_profile_hook() is None:
            if "/root/.axon_site" not in sys.path:
                sys.path.append("/root/.axon_site")
            from trn_agent_boot.trn_boot import _ntff_profile_via_ctypes

            hook = _ntff_profile_via_ctypes("/opt/axon/libaxon_pjrt.so")
            if hook is not None:
                ah.set_axon_ntff_profile_hook(hook)
    except Exception:
        pass
    try:
        import concourse.bass_utils as bu

        bu.upload_artifacts = lambda tmpdir: tmpdir
    except Exception:
        pass


def kernel(hidden_states, Wq, Wk, Wv, Wo):
    global LAST_EXEC_TIME_NS, LAST_RESULT
    hidden_states = np.asarray(hidden_states, dtype=np.float32)
    Wq, Wk, Wv, Wo = (np.asarray(w, dtype=np.float32) for w in (Wq, Wk, Wv, Wo))

    trace = bool(os.environ.get("BASS_TRACE"))
    if trace:
        _ensure_profile_support()
    nc = get_nc()
    maps = _in_maps(hidden_states, Wq, Wk, Wv, Wo)
    res = run_bass_kernel_spmd(
        nc,
        maps,
        core_ids=list(range(N_CORES)),
        trace=trace,
        tmpdir=os.environ.get("BASS_TRACE_DIR") or None,
    )
    LAST_RESULT = res
    LAST_EXEC_TIME_NS = res.exec_time_ns

    out = np.empty((B, S, DM), dtype=np.float32)
    for b in range(B):
        out[b] = res.results[2 * b]["out"] + res.results[2 * b + 1]["out"]
    return out


if __name__ == "__main__":
    rng = np.random.default_rng(0)
    hs = rng.standard_normal((B, S, DM), dtype=np.float32)
    ws = [
        (rng.standard_normal((DM, DM), dtype=np.float32) / np.sqrt(DM))
        for _ in range(4)
    ]
    o = kernel(hs, *ws)
    print("out", o.shape, o.dtype, float(np.abs(o).mean()))
    print("exec_time_ns", LAST_EXEC_TIME_NS)
